# revision 1
# baseline (speedup 1.0000x reference)
"""Trainium2 Bass kernel for the IoU polygon loss (nn_IoUPolyLoss).

Full inputs in, full (scalar) output out. Internally shards the 512
polygons over 8 NeuronCores (64 each: core c -> batch c//2, k-range
64*(c%2)). Each core rasterization-free computes, per polygon and
scanline, the coverage of pred/gt/xor regions via the alternating sum
of sorted edge-crossing counts; host combines the per-polygon integer
areas into the final loss:
    inter = (area_p + area_g - area_xor) / 2
    union = (area_p + area_g + area_xor) / 2

Device layout per core: partition p = hh*64 + poly (h-half hh), free
dims (side s=2, edge v=16, hl=64); scanline h = hh*64 + hl.
"""
import sys

import numpy as np

try:
    import concourse.bass as bass
except ImportError:
    sys.path.insert(0, "/opt/trn_rl_repo")
    import concourse.bass as bass

import concourse.mybir as mybir
import concourse.tile as tile
import concourse.bacc as bacc
from concourse.bass_utils import run_bass_kernel_spmd

OP = mybir.AluOpType
F32 = mybir.dt.float32
I32 = mybir.dt.int32
F = np.float32

MAGIC = 12582912.0           # 1.5 * 2^23, RN-to-int trick for |x| < 2^22
KK = 0.49545454545454547     # 0.5 - 1/220 margin

N_CORES = 8

LAST_RESULTS = None          # BassKernelResults of the most recent run


def _batcher16_pairs():
    n = 16
    rounds = []
    p = 1
    while p < n:
        k = p
        while k >= 1:
            los = []
            j = k % p
            while j <= n - 1 - k:
                for i in range(0, min(k, n - j - k)):
                    if (i + j) // (2 * p) == (i + j + k) // (2 * p):
                        los.append(i + j)
                j += 2 * k
            rounds.append((k, los))
            k //= 2
        p *= 2
    return rounds


def _decompose(idxs):
    n = len(idxs)
    if n == 1:
        return [[1, 1]]
    d = idxs[1] - idxs[0]
    if all(idxs[i] == idxs[0] + i * d for i in range(n)):
        return [[d, n]]
    run = 1
    while run < n and idxs[run] == idxs[0] + run * d:
        run += 1
    assert n % run == 0, f"cannot decompose {idxs}"
    outer = idxs[::run]
    do = outer[1] - outer[0]
    for oi, o in enumerate(outer):
        assert o == outer[0] + oi * do
        for ii in range(run):
            assert idxs[oi * run + ii] == o + ii * d, f"cannot decompose {idxs}"
    return [[do, len(outer)], [d, run]]


def _view(tile_ap, offset, dims):
    return bass.AP(
        tile_ap.tensor,
        tile_ap.offset + offset,
        [list(tile_ap.ap[0])] + [[s, c] for s, c in dims],
    )


def _build_core_kernel(tc, areas_dram, table, pidx, gverts):
    nc = tc.nc
    view = _view
    with tc.tile_pool(name="main", bufs=1) as pool:
        # loads
        pidx_sb = pool.tile([128, 1], I32, tag="pidx")
        nc.sync.dma_start(out=pidx_sb[:], in_=pidx)
        rawp = pool.tile([128, 32], F32, tag="rawp")
        nc.gpsimd.indirect_dma_start(
            out=rawp[:],
            out_offset=None,
            in_=table,
            in_offset=bass.IndirectOffsetOnAxis(ap=pidx_sb[:, :1], axis=0),
        )
        rawg = pool.tile([128, 32], F32, tag="rawg")
        nc.sync.dma_start(out=rawg[:], in_=gverts)
        # join both halves on the DVE so downstream ops have same-engine
        # deps only (HW limits sync-wait commands per instruction)
        raw = pool.tile([128, 64], F32, tag="raw")   # (s, v, coord)
        nc.vector.tensor_copy(raw[:, 0:32], rawp[:])
        nc.vector.tensor_copy(raw[:, 32:64], rawg[:])

        # exact trunc(x) + 100
        t0 = pool.tile([128, 64], F32, tag="t0")
        nc.vector.tensor_scalar(t0[:], raw[:], MAGIC, MAGIC, OP.add, OP.subtract)
        d1 = pool.tile([128, 64], F32, tag="d1")
        nc.vector.tensor_tensor(d1[:], t0[:], raw[:], OP.is_gt)
        d2 = pool.tile([128, 64], F32, tag="d2")
        nc.vector.tensor_tensor(d2[:], t0[:], raw[:], OP.is_lt)
        sp = pool.tile([128, 64], F32, tag="sp")
        nc.vector.tensor_scalar(sp[:], raw[:], 0.0, None, OP.is_ge)
        sn = pool.tile([128, 64], F32, tag="sn")
        nc.vector.tensor_scalar(sn[:], sp[:], -1.0, 1.0, OP.mult, OP.add)
        da = pool.tile([128, 64], F32, tag="da")
        nc.vector.tensor_tensor(da[:], d1[:], sp[:], OP.mult)
        db = pool.tile([128, 64], F32, tag="db")
        nc.vector.tensor_tensor(db[:], d2[:], sn[:], OP.mult)
        corr = pool.tile([128, 64], F32, tag="corr")
        nc.vector.tensor_tensor(corr[:], db[:], da[:], OP.subtract)
        t0p = pool.tile([128, 64], F32, tag="t0p")
        nc.vector.tensor_scalar(t0p[:], t0[:], 100.0, None, OP.add)
        verts = pool.tile([128, 64], F32, tag="verts")   # (s, v, coord)
        nc.vector.tensor_tensor(verts[:], t0p[:], corr[:], OP.add)

        x1v = view(verts[:], 0, [(32, 2), (2, 16)])
        y1v = view(verts[:], 1, [(32, 2), (2, 16)])

        x2 = pool.tile([128, 32], F32, tag="x2")
        x2v = view(x2[:], 0, [(16, 2), (1, 16)])
        nc.vector.tensor_copy(view(x2[:], 0, [(16, 2), (1, 15)]),
                              view(verts[:], 2, [(32, 2), (2, 15)]))
        nc.vector.tensor_copy(view(x2[:], 15, [(16, 2), (1, 1)]),
                              view(verts[:], 0, [(32, 2), (2, 1)]))
        y2 = pool.tile([128, 32], F32, tag="y2")
        y2v = view(y2[:], 0, [(16, 2), (1, 16)])
        nc.vector.tensor_copy(view(y2[:], 0, [(16, 2), (1, 15)]),
                              view(verts[:], 3, [(32, 2), (2, 15)]))
        nc.vector.tensor_copy(view(y2[:], 15, [(16, 2), (1, 1)]),
                              view(verts[:], 1, [(32, 2), (2, 1)]))

        dx = pool.tile([128, 32], F32, tag="dx")
        nc.vector.tensor_tensor(view(dx[:], 0, [(16, 2), (1, 16)]), x2v, x1v,
                                OP.subtract)
        dy = pool.tile([128, 32], F32, tag="dy")
        nc.vector.tensor_tensor(view(dy[:], 0, [(16, 2), (1, 16)]), y2v, y1v,
                                OP.subtract)
        z = pool.tile([128, 32], F32, tag="z")
        nc.vector.tensor_scalar(z[:], dy[:], 0.0, None, OP.is_equal)
        ds = pool.tile([128, 32], F32, tag="ds")
        nc.vector.tensor_tensor(ds[:], dy[:], z[:], OP.add)
        rt = pool.tile([128, 32], F32, tag="rt")
        nc.vector.reciprocal(rt[:], ds[:])

        # py = hh*64 + hl
        hlq = pool.tile([128, 64], I32, tag="hlq")
        nc.gpsimd.iota(hlq[:], pattern=[[1, 64]], base=0, channel_multiplier=0)
        pid = pool.tile([128, 1], I32, tag="pid")
        nc.gpsimd.iota(pid[:], pattern=[[0, 1]], base=0, channel_multiplier=1)
        hh64 = pool.tile([128, 1], I32, tag="hh64")
        nc.vector.tensor_scalar(hh64[:], pid[:], 64, None, OP.bitwise_and)
        hh64f = pool.tile([128, 1], F32, tag="hh64f")
        nc.vector.tensor_copy(hh64f[:], hh64[:])
        hlf = pool.tile([128, 64], F32, tag="hlf")
        nc.vector.tensor_copy(hlf[:], hlq[:])
        pyf = pool.tile([128, 64], F32, tag="pyf")
        nc.vector.tensor_scalar(pyf[:], hlf[:], hh64f[:, :1], None, OP.add)

        # grid [128, (s=2, v=16, hl=64)]
        GD = [(1024, 2), (64, 16), (1, 64)]
        y1g = view(verts[:], 1, [(32, 2), (2, 16), (0, 64)])
        y2g = view(y2[:], 0, [(16, 2), (1, 16), (0, 64)])
        x1g = view(verts[:], 0, [(32, 2), (2, 16), (0, 64)])
        dxg = view(dx[:], 0, [(16, 2), (1, 16), (0, 64)])
        rtg = view(rt[:], 0, [(16, 2), (1, 16), (0, 64)])
        pyg = view(pyf[:], 0, [(0, 2), (0, 16), (1, 64)])

        def gt_tile(tag):
            t = pool.tile([128, 2048], F32, tag=tag)
            return t, view(t[:], 0, GD)

        ta, ag_ = gt_tile("ga")
        nc.vector.tensor_tensor(ag_, y1g, pyg, OP.is_le)
        tb, bg = gt_tile("gb")
        nc.vector.tensor_tensor(bg, y2g, pyg, OP.is_le)
        tcr, crg = gt_tile("gcr")
        nc.vector.tensor_tensor(crg, ag_, bg, OP.not_equal)
        tn, ng = gt_tile("gn")
        nc.vector.tensor_tensor(ng, pyg, y1g, OP.subtract)
        tt_, ttg = gt_tile("gtt")
        nc.vector.tensor_tensor(ttg, ng, rtg, OP.mult)
        tu, ug = gt_tile("gu")
        nc.vector.tensor_tensor(ug, ttg, dxg, OP.mult)
        txi, xig = gt_tile("gxi")
        nc.vector.tensor_tensor(xig, ug, x1g, OP.add)
        tw, wg = gt_tile("gw")
        nc.vector.tensor_scalar(wg, xig, KK, MAGIC, OP.add, OP.add)
        tw2, w2g = gt_tile("gw2")
        nc.vector.tensor_scalar(w2g, wg, MAGIC, None, OP.subtract)
        tc_, cg = gt_tile("gc")
        nc.vector.tensor_tensor(cg, w2g, crg, OP.mult)

        # sort-16 along v (ascending), in place
        for k, los in _batcher16_pairs():
            dims = _decompose(los)
            npairs = len(los)
            lo_ap = view(tc_[:], los[0] * 64,
                         [(1024, 2)] + [[s * 64, c] for s, c in dims] + [(1, 64)])
            hi_ap = view(tc_[:], (los[0] + k) * 64,
                         [(1024, 2)] + [[s * 64, c] for s, c in dims] + [(1, 64)])
            tmp = pool.tile([128, 2 * npairs * 64], F32, tag="sorttmp")
            tdims = [(npairs * 64, 2)] + (
                [[dims[1][1] * 64, dims[0][1]], [64, dims[1][1]]]
                if len(dims) == 2 else [[64, dims[0][1]]]
            ) + [(1, 64)]
            tmp_ap = view(tmp[:], 0, tdims)
            nc.vector.tensor_tensor(tmp_ap, lo_ap, hi_ap, OP.max)
            nc.vector.tensor_tensor(lo_ap, lo_ap, hi_ap, OP.min)
            nc.vector.tensor_copy(hi_ap, tmp_ap)

        # merge-32: [pred asc] ++ [gt desc] then bitonic merge
        tm = pool.tile([128, 2048], F32, tag="gm")   # (v32, hl)
        nc.vector.tensor_copy(view(tm[:], 0, [(64, 16), (1, 64)]),
                              view(tc_[:], 0, [(64, 16), (1, 64)]))
        nc.vector.tensor_copy(view(tm[:], 16 * 64, [(64, 16), (1, 64)]),
                              view(tc_[:], 1024 + 15 * 64, [(-64, 16), (1, 64)]))
        for d in (16, 8, 4, 2, 1):
            lo_ap = view(tm[:], 0, [(2 * d * 64, 16 // d), (64, d), (1, 64)])
            hi_ap = view(tm[:], d * 64, [(2 * d * 64, 16 // d), (64, d), (1, 64)])
            tmp = pool.tile([128, 1024], F32, tag="mergetmp")
            tmp_ap = view(tmp[:], 0, [(d * 64, 16 // d), (64, d), (1, 64)])
            nc.vector.tensor_tensor(tmp_ap, lo_ap, hi_ap, OP.max)
            nc.vector.tensor_tensor(lo_ap, lo_ap, hi_ap, OP.min)
            nc.vector.tensor_copy(hi_ap, tmp_ap)

        # alternating dots (ascending: +1 at odd 0-based v)
        sg16i = pool.tile([128, 16], I32, tag="sg16i")
        nc.gpsimd.iota(sg16i[:], pattern=[[1, 16]], base=0, channel_multiplier=0)
        sg16m = pool.tile([128, 16], I32, tag="sg16m")
        nc.vector.tensor_scalar(sg16m[:], sg16i[:], 1, None, OP.bitwise_and)
        sg16f = pool.tile([128, 16], F32, tag="sg16f")
        nc.vector.tensor_copy(sg16f[:], sg16m[:])
        sg16 = pool.tile([128, 16], F32, tag="sg16")
        nc.vector.tensor_scalar(sg16[:], sg16f[:], 2.0, -1.0, OP.mult, OP.add)

        sg32i = pool.tile([128, 32], I32, tag="sg32i")
        nc.gpsimd.iota(sg32i[:], pattern=[[1, 32]], base=0, channel_multiplier=0)
        sg32m = pool.tile([128, 32], I32, tag="sg32m")
        nc.vector.tensor_scalar(sg32m[:], sg32i[:], 1, None, OP.bitwise_and)
        sg32f = pool.tile([128, 32], F32, tag="sg32f")
        nc.vector.tensor_copy(sg32f[:], sg32m[:])
        sg32 = pool.tile([128, 32], F32, tag="sg32")
        nc.vector.tensor_scalar(sg32[:], sg32f[:], 2.0, -1.0, OP.mult, OP.add)

        tsp, spg = gt_tile("gsp")
        nc.vector.tensor_tensor(
            spg, cg, view(sg16[:], 0, [(0, 2), (1, 16), (0, 64)]), OP.mult)
        a16 = pool.tile([128, 128], F32, tag="a16")   # (s, hl)
        nc.vector.tensor_reduce(view(a16[:], 0, [(64, 2), (1, 64)]),
                                view(tsp[:], 0, [(1024, 2), (1, 64), (64, 16)]),
                                axis=mybir.AxisListType.X, op=OP.add)
        a2 = pool.tile([128, 2], F32, tag="a2")       # (s,)
        nc.vector.tensor_reduce(view(a2[:], 0, [(1, 2)]),
                                view(a16[:], 0, [(64, 2), (1, 64)]),
                                axis=mybir.AxisListType.X, op=OP.add)

        tsm = pool.tile([128, 2048], F32, tag="gsm")
        nc.vector.tensor_tensor(view(tsm[:], 0, [(64, 32), (1, 64)]),
                                view(tm[:], 0, [(64, 32), (1, 64)]),
                                view(sg32[:], 0, [(1, 32), (0, 64)]), OP.mult)
        ax64 = pool.tile([128, 64], F32, tag="ax64")  # (hl,)
        nc.vector.tensor_reduce(view(ax64[:], 0, [(1, 64)]),
                                view(tsm[:], 0, [(1, 64), (64, 32)]),
                                axis=mybir.AxisListType.X, op=OP.add)
        ax = pool.tile([128, 1], F32, tag="ax")
        nc.vector.tensor_reduce(ax[:, 0:1], ax64[:, 0:64],
                                axis=mybir.AxisListType.X, op=OP.add)

        # assemble [128, 3] (h-half partials; host sums p and p+64)
        ar = pool.tile([128, 3], F32, tag="ar")
        nc.vector.tensor_copy(ar[:, 0:2], a2[:, 0:2])
        nc.vector.tensor_copy(ar[:, 2:3], ax[:, 0:1])
        nc.sync.dma_start(out=areas_dram, in_=ar[:])


_CACHED_NC = None


def _get_nc():
    global _CACHED_NC
    if _CACHED_NC is not None:
        return _CACHED_NC
    nc = bacc.Bacc("TRN2", target_bir_lowering=False, debug=False,
                   num_devices=N_CORES)
    table = nc.dram_tensor("table", [16384, 32], F32, kind="ExternalInput")
    pidx = nc.dram_tensor("pidx", [128, 1], I32, kind="ExternalInput")
    gverts = nc.dram_tensor("gverts", [128, 32], F32, kind="ExternalInput")
    areas = nc.dram_tensor("areas", [128, 3], F32, kind="ExternalOutput")
    with tile.TileContext(nc) as tc:
        _build_core_kernel(tc, areas.ap(), table.ap(), pidx.ap(), gverts.ap())
    nc.compile()
    _CACHED_NC = nc
    return nc


def kernel(output, mask, ind, target):
    global LAST_RESULTS
    output = np.asarray(output)
    mask = np.asarray(mask)
    ind = np.asarray(ind)
    target = np.asarray(target)
    B, C, H, W = output.shape

    # ---- host-side sharding (layout-only)
    in_maps = []
    for c in range(N_CORES):
        b, k0 = c // 2, 64 * (c % 2)
        table = np.ascontiguousarray(output[b].reshape(C, H * W).T).astype(F)
        idx64 = ind[b, k0:k0 + 64].astype(np.int32)
        pidx = np.tile(idx64, 2).reshape(128, 1)
        gv64 = np.ascontiguousarray(target[b, :, k0:k0 + 64].T).astype(F)
        gverts = np.tile(gv64, (2, 1))
        in_maps.append({"table": table, "pidx": pidx, "gverts": gverts})

    nc = _get_nc()
    res = run_bass_kernel_spmd(nc, in_maps, core_ids=list(range(N_CORES)))
    LAST_RESULTS = res

    # ---- host-side gather + final scalar assembly
    areas = np.zeros((B, 128, 3), np.float32)
    for c in range(N_CORES):
        b, k0 = c // 2, 64 * (c % 2)
        halves = res.results[c]["areas"]
        areas[b, k0:k0 + 64] = halves[:64] + halves[64:]
    ap, ag, ax = areas[..., 0], areas[..., 1], areas[..., 2]
    inter = ((ap + ag - ax) / 2).astype(F)
    union = ((ap + ag + ax) / 2).astype(F)
    iou = (inter / (union + F(1e-4))).astype(F)
    m = mask.astype(F)
    loss = F(F(1.0) - (iou * m).sum(dtype=F) / (m.sum(dtype=F) + F(1e-4)))
    return np.asarray(loss, dtype=np.float32)



# revision 13
# speedup vs baseline: 1.7155x; 1.7155x over previous
"""Trainium2 Bass kernel for the IoU polygon loss (nn_IoUPolyLoss).

Full inputs in, full (scalar) output out. Internally shards the 512
polygons over 8 NeuronCores (64 each: core c -> batch c//2, k-range
64*(c%2)). Rasterization-free: per polygon and scanline, coverage is
the alternating sum of the sorted edge-crossing x-coordinates; the
host combines per-polygon areas into the final loss:
    inter = (area_p + area_g - area_xor) / 2
    union = (area_p + area_g + area_xor) / 2

Device layout per core: partition p = hh*64 + poly (h-half hh), free
dims (side s=2, edge v=16, hl=64); scanline h = hh*64 + hl.

v2 design notes (cost-model driven):
 - affine crossing form xint = py*A + B with per-edge A = dx/dy,
   B = x1 - y1*A (tiny precompute) -> 4 big DVE ops instead of 7
 - crossing mask on the Pool engine in parallel with the DVE chain,
   using bg = roll(ag) (y2 of edge v is y1 of edge v+1)
 - values shifted by -128 before sorting: masked slots (0.0) sort
   ABOVE all real crossings (which are negative) and cancel pairwise
   under the alternating signs, so no sentinel fixup is needed
 - sort-16 / merge-32 in fp16 (integers < 2048 are exact): 2x DVE
   throughput on min/max, 4x on copies; full-coverage rounds are
   ping-ponged (2 ops), sparse rounds run in place (3 ops)
 - final alternating sums via scalar_tensor_tensor accum_out: one
   instruction per area (host only ever needs ap+ag and ax)
"""
import sys

import numpy as np

try:
    import concourse.bass as bass
except ImportError:
    sys.path.insert(0, "/opt/trn_rl_repo")
    import concourse.bass as bass

import concourse.mybir as mybir
import concourse.tile as tile
import concourse.bacc as bacc
from concourse.bass_utils import run_bass_kernel_spmd

OP = mybir.AluOpType
F32 = mybir.dt.float32
F16 = mybir.dt.float16
I32 = mybir.dt.int32
F = np.float32

MAGIC = 12582912.0            # 1.5 * 2^23, RN-to-int trick for |x| < 2^22
KK = 0.49545454545454547      # 0.5 - 1/220 margin
M2 = MAGIC + 128.0            # exact in fp32
MAGICM100 = MAGIC - 100.0     # exact in fp32

N_CORES = 8

LAST_RESULTS = None           # BassKernelResults of the most recent run


def _batcher16_pairs():
    n = 16
    rounds = []
    p = 1
    while p < n:
        k = p
        while k >= 1:
            los = []
            j = k % p
            while j <= n - 1 - k:
                for i in range(0, min(k, n - j - k)):
                    if (i + j) // (2 * p) == (i + j + k) // (2 * p):
                        los.append(i + j)
                j += 2 * k
            rounds.append((k, los))
            k //= 2
        p *= 2
    return rounds


def _decompose(idxs):
    n = len(idxs)
    if n == 1:
        return [[1, 1]]
    d = idxs[1] - idxs[0]
    if all(idxs[i] == idxs[0] + i * d for i in range(n)):
        return [[d, n]]
    run = 1
    while run < n and idxs[run] == idxs[0] + run * d:
        run += 1
    assert n % run == 0, f"cannot decompose {idxs}"
    outer = idxs[::run]
    do = outer[1] - outer[0]
    for oi, o in enumerate(outer):
        assert o == outer[0] + oi * do
        for ii in range(run):
            assert idxs[oi * run + ii] == o + ii * d, f"cannot decompose {idxs}"
    return [[do, len(outer)], [d, run]]


def _view(tile_ap, offset, dims):
    return bass.AP(
        tile_ap.tensor,
        tile_ap.offset + offset,
        [list(tile_ap.ap[0])] + [[s, c] for s, c in dims],
    )


def _vdims(idxs, inner=64):
    """AP dims for a set of v-indices (times stride 64, hl inner)."""
    return [[s * 64, c] for s, c in _decompose(idxs)] + [[1, inner]]


def _build_core_kernel(tc, areas_dram, table, pidx, gverts):
    nc = tc.nc
    view = _view
    with tc.tile_pool(name="main", bufs=1) as pool:
        # ---------------- loads
        pidx_sb = pool.tile([128, 1], I32, tag="pidx")
        nc.sync.dma_start(out=pidx_sb[:], in_=pidx)
        rawp = pool.tile([128, 32], F32, tag="rawp")
        nc.gpsimd.indirect_dma_start(
            out=rawp[:],
            out_offset=None,
            in_=table,
            in_offset=bass.IndirectOffsetOnAxis(ap=pidx_sb[:, :1], axis=0),
        )
        rawg = pool.tile([128, 32], F32, tag="rawg")
        nc.sync.dma_start(out=rawg[:], in_=gverts)
        # join both halves on the DVE (same-engine deps downstream)
        raw = pool.tile([128, 64], F32, tag="raw")   # (s, v, coord)
        nc.vector.tensor_copy(raw[:, 0:32], rawp[:])
        nc.vector.tensor_copy(raw[:, 32:64], rawg[:])

        # ---------------- trunc(x)+100 via round(x - 0.5*sgn(x)) + 100
        m = pool.tile([128, 64], F32, tag="m")
        nc.vector.tensor_scalar(m[:], raw[:], 0.0, None, OP.is_ge)
        sh = pool.tile([128, 64], F32, tag="sh")
        nc.vector.tensor_scalar(sh[:], m[:], -1.0, 0.5, OP.mult, OP.add)
        u = pool.tile([128, 64], F32, tag="u")
        nc.vector.tensor_tensor(u[:], raw[:], sh[:], OP.add)
        verts = pool.tile([128, 64], F32, tag="verts")   # (s, v, coord)
        nc.vector.tensor_scalar(verts[:], u[:], MAGIC, MAGICM100,
                                OP.add, OP.subtract)

        # ---------------- edge precompute: A = dx/dy', B = x1 - y1*A
        ver2 = pool.tile([128, 64], F32, tag="ver2")     # roll v by 1
        nc.vector.tensor_copy(view(ver2[:], 0, [(32, 2), (2, 15), (1, 2)]),
                              view(verts[:], 2, [(32, 2), (2, 15), (1, 2)]))
        nc.vector.tensor_copy(view(ver2[:], 30, [(32, 2), (1, 2)]),
                              view(verts[:], 0, [(32, 2), (1, 2)]))
        dall = pool.tile([128, 64], F32, tag="dall")     # (dx, dy) per edge
        nc.vector.tensor_tensor(dall[:], ver2[:], verts[:], OP.subtract)

        pk = [(16, 2), (1, 16)]          # packed (s, v) dims for [128,32]
        x1v = view(verts[:], 0, [(32, 2), (2, 16)])
        y1v = view(verts[:], 1, [(32, 2), (2, 16)])
        dxv = view(dall[:], 0, [(32, 2), (2, 16)])
        dyv = view(dall[:], 1, [(32, 2), (2, 16)])

        z = pool.tile([128, 32], F32, tag="z")
        nc.vector.tensor_scalar(view(z[:], 0, pk), dyv, 0.0, None, OP.is_equal)
        ds = pool.tile([128, 32], F32, tag="ds")
        nc.vector.tensor_tensor(view(ds[:], 0, pk), dyv, view(z[:], 0, pk),
                                OP.add)
        rt = pool.tile([128, 32], F32, tag="rt")
        nc.vector.reciprocal(rt[:], ds[:])
        At = pool.tile([128, 32], F32, tag="At")
        nc.vector.tensor_tensor(view(At[:], 0, pk),
                                view(rt[:], 0, pk), dxv, OP.mult)
        n1 = pool.tile([128, 32], F32, tag="n1")
        nc.vector.tensor_tensor(view(n1[:], 0, pk), y1v,
                                view(At[:], 0, pk), OP.mult)
        Bt = pool.tile([128, 32], F32, tag="Bt")
        nc.vector.scalar_tensor_tensor(view(Bt[:], 0, pk),
                                       view(n1[:], 0, pk), -1.0, x1v,
                                       OP.mult, OP.add)

        # ---------------- py = hh*64 + hl  (f32 [128, 64])
        hlq = pool.tile([128, 64], I32, tag="hlq")
        nc.gpsimd.iota(hlq[:], pattern=[[1, 64]], base=0, channel_multiplier=0)
        pid = pool.tile([128, 1], I32, tag="pid")
        nc.gpsimd.iota(pid[:], pattern=[[0, 1]], base=0, channel_multiplier=1)
        hh64 = pool.tile([128, 1], I32, tag="hh64")
        nc.vector.tensor_scalar(hh64[:], pid[:], 64, None, OP.bitwise_and)
        hh64f = pool.tile([128, 1], F32, tag="hh64f")
        nc.vector.tensor_copy(hh64f[:], hh64[:])
        hlf = pool.tile([128, 64], F32, tag="hlf")
        nc.vector.tensor_copy(hlf[:], hlq[:])
        pyf = pool.tile([128, 64], F32, tag="pyf")
        nc.vector.tensor_scalar(pyf[:], hlf[:], hh64f[:, :1], None, OP.add)

        # fp16 casts of y1 and py for the crossing-mask compares
        y1h = pool.tile([128, 32], F16, tag="y1h")
        nc.vector.tensor_copy(view(y1h[:], 0, pk), y1v)
        pyh = pool.tile([128, 64], F16, tag="pyh")
        nc.vector.tensor_copy(pyh[:], pyf[:])

        # ---------------- grid stage, free = (s2, v16, hl64) = 2048
        GD = [(1024, 2), (64, 16), (1, 64)]
        Ag = view(At[:], 0, [(16, 2), (1, 16), (0, 64)])
        pyg = view(pyf[:], 0, [(0, 2), (0, 16), (1, 64)])
        pyg16 = view(pyh[:], 0, [(0, 2), (0, 16), (1, 64)])
        y1g16 = view(y1h[:], 0, [(16, 2), (1, 16), (0, 64)])

        def gtile(tag, dt=F32):
            t = pool.tile([128, 2048], dt, tag=tag)
            return t, view(t[:], 0, GD)

        # flat (s*v, hl) views: scalar_tensor_tensor needs <=2 free dims
        GF = [(64, 32), (1, 64)]
        BF = [(1, 32), (0, 64)]              # per-edge smalls broadcast on hl

        # DVE: t1 = py*A ; xkk = (t1+KK)+B.   Act: wr = xkk+MAGIC.
        t1, t1g = gtile("t1")
        nc.vector.tensor_tensor(t1g, pyg, Ag, OP.mult)
        xkk, xkkg = gtile("xkk")
        nc.vector.scalar_tensor_tensor(view(xkk[:], 0, GF), view(t1[:], 0, GF),
                                       KK, view(Bt[:], 0, BF), OP.add, OP.add)
        wr, wrg = gtile("wr")
        nc.scalar.activation(view(wr[:], 0, GF), view(xkk[:], 0, GF),
                             mybir.ActivationFunctionType.Copy, bias=MAGIC)

        # DVE fp16 (overlaps Act): ag = (y1 <= py); crg = ag != roll(ag)
        agt, agg = gtile("ag", F16)
        nc.vector.tensor_tensor(agg, y1g16, pyg16, OP.is_le)
        crt, crg = gtile("cr", F16)
        nc.vector.tensor_tensor(view(crt[:], 0, [(1024, 2), (64, 15), (1, 64)]),
                                view(agt[:], 64, [(1024, 2), (64, 15), (1, 64)]),
                                view(agt[:], 0, [(1024, 2), (64, 15), (1, 64)]),
                                OP.not_equal)
        nc.vector.tensor_tensor(view(crt[:], 15 * 64, [(1024, 2), (1, 64)]),
                                view(agt[:], 0, [(1024, 2), (1, 64)]),
                                view(agt[:], 15 * 64, [(1024, 2), (1, 64)]),
                                OP.not_equal)

        # join: cg = (wr - (MAGIC+128)) * crg  -> fp16, shifted by -128
        T0, T0g = gtile("T0", F16)
        T1, T1g = gtile("T1", F16)
        nc.vector.scalar_tensor_tensor(view(T0[:], 0, GF), view(wr[:], 0, GF),
                                       M2, view(crt[:], 0, GF),
                                       OP.subtract, OP.mult)

        # ---------------- sort-16 along v (ascending), fp16
        # ping-pong when idle < np, else in-place; track current buffer
        tmp16 = pool.tile([128, 1024], F16, tag="tmp16")
        bufs = [T0, T1]
        cur = 0
        for k, los in _batcher16_pairs():
            npairs = len(los)
            touched = sorted(los + [l + k for l in los])
            idle = [i for i in range(16) if i not in touched]
            C = bufs[cur]
            lo_dims = [(1024, 2)] + _vdims(los)
            lo_src = view(C[:], los[0] * 64, lo_dims)
            hi_src = view(C[:], (los[0] + k) * 64, lo_dims)
            if len(idle) < npairs:          # ping-pong round
                N = bufs[1 - cur]
                nc.vector.tensor_tensor(view(N[:], los[0] * 64, lo_dims),
                                        lo_src, hi_src, OP.min)
                nc.vector.tensor_tensor(view(N[:], (los[0] + k) * 64, lo_dims),
                                        lo_src, hi_src, OP.max)
                if idle:
                    # off the DVE: Act copies the untouched lanes in parallel
                    idims = [(1024, 2)] + _vdims(idle)
                    nc.scalar.activation(view(N[:], idle[0] * 64, idims),
                                         view(C[:], idle[0] * 64, idims),
                                         mybir.ActivationFunctionType.Copy)
                cur = 1 - cur
            else:                            # in-place round
                dd = _decompose(los)
                tdims = [(npairs * 64, 2)] + (
                    [[dd[1][1] * 64, dd[0][1]], [64, dd[1][1]]]
                    if len(dd) == 2 else [[64, dd[0][1]]]
                ) + [(1, 64)]
                tmp_ap = view(tmp16[:], 0, tdims)
                nc.vector.tensor_tensor(tmp_ap, lo_src, hi_src, OP.max)
                nc.vector.tensor_tensor(lo_src, lo_src, hi_src, OP.min)
                nc.vector.tensor_copy(hi_src, tmp_ap)
        S = bufs[cur]                        # sorted, fp16, (s, v16, hl)

        # ---------------- area_p + area_g on Act: odd-rank sum minus
        # even-rank sum (overlaps with the merge running on the DVE)
        scr1 = pool.tile([128, 2048], F16, tag="scr1")
        arO = pool.tile([128, 4], F32, tag="arO")    # odd/even partials
        ODD = [(128, 16), (1, 64)]                    # every 2nd sv-lane
        nc.scalar.activation(view(scr1[:], 64, ODD), view(S[:], 64, ODD),
                             mybir.ActivationFunctionType.Copy,
                             accum_out=arO[:, 0:1])
        nc.scalar.activation(view(scr1[:], 0, ODD), view(S[:], 0, ODD),
                             mybir.ActivationFunctionType.Copy,
                             accum_out=arO[:, 1:2])

        # ---------------- merge-32: [pred asc ++ gt desc], bitonic, fp16
        M0 = pool.tile([128, 2048], F16, tag="M0")   # (v32, hl)
        M1 = pool.tile([128, 2048], F16, tag="M1")
        nc.vector.tensor_copy(view(M0[:], 0, [(64, 16), (1, 64)]),
                              view(S[:], 0, [(64, 16), (1, 64)]))
        nc.vector.tensor_copy(view(M0[:], 16 * 64, [(64, 16), (1, 64)]),
                              view(S[:], 1024 + 15 * 64, [(-64, 16), (1, 64)]))
        mbufs = [M0, M1]
        mcur = 0
        for d in (16, 8, 4, 2, 1):
            C = mbufs[mcur]
            N = mbufs[1 - mcur]
            dims = [(2 * d * 64, 16 // d), (64, d), (1, 64)]
            lo_src = view(C[:], 0, dims)
            hi_src = view(C[:], d * 64, dims)
            nc.vector.tensor_tensor(view(N[:], 0, dims), lo_src, hi_src,
                                    OP.min)
            nc.vector.tensor_tensor(view(N[:], d * 64, dims), lo_src, hi_src,
                                    OP.max)
            mcur = 1 - mcur
        M = mbufs[mcur]

        # ---------------- area_xor: odd ranks on Act, even ranks on DVE
        scr2 = pool.tile([128, 1024], F16, tag="scr2")
        MODD = [(128, 16), (1, 64)]
        nc.scalar.activation(view(scr2[:], 0, [(64, 16), (1, 64)]),
                             view(M[:], 64, MODD),
                             mybir.ActivationFunctionType.Copy,
                             accum_out=arO[:, 2:3])
        nc.vector.tensor_reduce(arO[:, 3:4], view(M[:], 0, MODD),
                                axis=mybir.AxisListType.XY, op=OP.add)

        # combine: ar[j] = odd[j] - even[j]
        ar = pool.tile([128, 2], F32, tag="ar")
        nc.vector.tensor_tensor(view(ar[:], 0, [(1, 2)]),
                                view(arO[:], 0, [(2, 2)]),
                                view(arO[:], 1, [(2, 2)]), OP.subtract)

        nc.sync.dma_start(out=areas_dram, in_=ar[:])


_CACHED_NC = None


def _get_nc():
    global _CACHED_NC
    if _CACHED_NC is not None:
        return _CACHED_NC
    nc = bacc.Bacc("TRN2", target_bir_lowering=False, debug=False,
                   num_devices=N_CORES)
    table = nc.dram_tensor("table", [16384, 32], F32, kind="ExternalInput")
    pidx = nc.dram_tensor("pidx", [128, 1], I32, kind="ExternalInput")
    gverts = nc.dram_tensor("gverts", [128, 32], F32, kind="ExternalInput")
    areas = nc.dram_tensor("areas", [128, 2], F32, kind="ExternalOutput")
    with tile.TileContext(nc) as tc:
        _build_core_kernel(tc, areas.ap(), table.ap(), pidx.ap(), gverts.ap())
    nc.compile()
    _CACHED_NC = nc
    return nc


def kernel(output, mask, ind, target):
    global LAST_RESULTS
    output = np.asarray(output)
    mask = np.asarray(mask)
    ind = np.asarray(ind)
    target = np.asarray(target)
    B, C, H, W = output.shape

    # ---- host-side sharding (layout-only)
    in_maps = []
    for c in range(N_CORES):
        b, k0 = c // 2, 64 * (c % 2)
        table = np.ascontiguousarray(output[b].reshape(C, H * W).T).astype(F)
        idx64 = ind[b, k0:k0 + 64].astype(np.int32)
        pidx = np.tile(idx64, 2).reshape(128, 1)
        gv64 = np.ascontiguousarray(target[b, :, k0:k0 + 64].T).astype(F)
        gverts = np.tile(gv64, (2, 1))
        in_maps.append({"table": table, "pidx": pidx, "gverts": gverts})

    nc = _get_nc()
    res = run_bass_kernel_spmd(nc, in_maps, core_ids=list(range(N_CORES)))
    LAST_RESULTS = res

    # ---- host-side gather + final scalar assembly
    spg = np.zeros((B, 128), np.float32)     # area_p + area_g per poly
    ax = np.zeros((B, 128), np.float32)      # area_xor per poly
    for c in range(N_CORES):
        b, k0 = c // 2, 64 * (c % 2)
        halves = res.results[c]["areas"]     # [128, 2]
        spg[b, k0:k0 + 64] = halves[:64, 0] + halves[64:, 0]
        ax[b, k0:k0 + 64] = halves[:64, 1] + halves[64:, 1]
    inter = ((spg - ax) / 2).astype(F)
    union = ((spg + ax) / 2).astype(F)
    iou = (inter / (union + F(1e-4))).astype(F)
    mm = mask.astype(F)
    loss = F(F(1.0) - (iou * mm).sum(dtype=F) / (mm.sum(dtype=F) + F(1e-4)))
    return np.asarray(loss, dtype=np.float32)
